# revision 1
# baseline (speedup 1.0000x reference)
"""GRU decoder (teacher forcing) + log_softmax on 8 Trainium2 NeuronCores.

v3 strategy (vocab-sharded projection, replicated recurrence):
  - Hidden state is step-major [128, S, KH, B]: per-step writes are fully
    contiguous.  HT (bf16) feeds the recurrence + elementwise path; HT8
    (fp8, written off-critical by DVE) feeds the phase-2 DoubleRow matmuls.
  - Recurrence matmuls are bf16 (48 small matmuls pipeline at ~25ns each,
    better than fp8 DoubleRow).  Weights are packed x64 so the fp8 x-side
    products match; ACT tanh scales divide it back out.
  - Phase 0 runs in 2-step chunks: one 128-row indirect gather + one
    DMA-transpose pair per 2 chunks, a DVE cast to fp8 (x32), then fp8
    DoubleRow x-gate matmuls written DIRECTLY into the PSUM banks that the
    per-step W_hh matmuls accumulate into (r/z one bank, xn another).
  - Per step (emitted in a high-priority band so the scheduler never
    wedges phase-2 work ahead of ready gate matmuls): PE r,z,n -> ACT
    tanh(r), tanh(z), q=1-z; DVE r_s, z_s, p_s, n-chain, h' (bf16) and
    the fp8 mirror copy.
  - Phase 2 (per 128-row tile, vocab units of 1000): fp8 DoubleRow logits
    into a 2-bank PSUM tile; ACT exp(logit-4ln2) with accum_out row sums;
    DVE stores logits/64 as f16 (two 500-col pieces).  Per stat group one
    tiny AllReduce; its result read is DEFERRED a fixed number of steps so
    no queue ever waits on collective latency (group 0 is a single row-
    tile and absorbs the ~60us collective bootstrap).  lse via DVE
    frexp+poly; final out = logit - lse on GpSimd in 500-col pieces; out
    DMA per 2000 cols on Sync.

kernel(**inputs) takes FULL numpy inputs, preps layouts on host, runs the
SPMD NEFF on cores 0..7 and reassembles the [32, 64, 32000] output.
"""

import os

import numpy as np
import ml_dtypes

import concourse.bass as bass
import concourse.bacc as bacc
import concourse.mybir as mybir
import concourse.tile as tile
from concourse.bass_utils import run_bass_kernel_spmd

# problem shape (hardcoded per contract)
B, T, V, E, H = 32, 64, 32000, 256, 512
S = T - 1                 # 63 decode steps
NCORES = 8
VS = V // NCORES          # 4000 vocab shard per core
G = 3 * H                 # 1536 gate dims
GC = G // 128             # 12 gate chunks
KH = H // 128             # 4 contraction tiles over H
KE = E // 128             # 2 contraction tiles over E
NROW = S * B              # 2016 output rows, (t, b) order
NMT = (NROW + 127) // 128  # 16 row-tiles (last has 96 rows)
CH = 2                    # recurrence steps per phase-0 chunk
NCH = (S + CH - 1) // CH  # 32 chunks (last has 1 step)
VU = 1000                 # vocab unit for psum/exp (2 psum banks)
NVU = VS // VU            # 4 units per row-tile
WS = 64.0                 # gate preact scale (fp8 headroom)
W_IH_S = 2.0              # W_ih fp8 scale;  x EMB_S = WS
EMB_S = 32.0              # embedding fp8 scale
LN2 = float(np.log(2.0))
EXP_BIAS = -4.0 * LN2     # exp(logit - 4ln2): keeps exp safely in range
# stat-collective groups (first mtile, n mtiles).  The first collective on
# this fabric pays a ~150-190us bootstrap, so a warmup AllReduce (that
# nothing depends on) is issued at t~0 and the first REAL collective fires
# only after it completes (~step 40); collectives cost ~10-20us wall each,
# so use only three.
GROUPS = [(0, 2), (2, 8), (10, 4), (14, 2)]
# steps of delay between issuing a group's AllReduce and reading its result
GROUP_DELAY = {0: 34, 1: 4}
GROUP_DELAY_DEFAULT = 4

F32 = mybir.dt.float32
BF16 = mybir.dt.bfloat16
F16 = mybir.dt.float16
F8 = mybir.dt.float8e4
I32 = mybir.dt.int32
U32 = mybir.dt.uint32
AF = mybir.ActivationFunctionType
OP = mybir.AluOpType
DR = mybir.MatmulPerfMode.DoubleRow

# -ln(m) Chebyshev-interpolation coefficients on m in [1, 2], highest first.
_nodes = np.cos((2 * np.arange(1, 7) - 1) / (2 * 6.0) * np.pi) * 0.5 + 1.5
_NEGLN_COEF = [float(c) for c in np.polyfit(_nodes, -np.log(_nodes), 5)]

_BUILD_CACHE = {}


def _build(bhn_nz: bool, bx_nz: bool, bproj_nz: bool):
    key = (bhn_nz, bx_nz, bproj_nz)
    if key in _BUILD_CACHE:
        return _BUILD_CACHE[key]

    nc = bacc.Bacc("TRN2", target_bir_lowering=False, debug=False,
                   enable_asserts=False, num_devices=NCORES)

    trg_d = nc.dram_tensor("trg_flat", (NROW, 1), I32, kind="ExternalInput")
    tbl_d = nc.dram_tensor("emb_tbl", (V, E), BF16, kind="ExternalInput")
    wih_d = nc.dram_tensor("wih8_t", (128, KE, G), F8, kind="ExternalInput")
    whh_d = nc.dram_tensor("whh_t", (128, KH, G), BF16, kind="ExternalInput")
    h0_d = nc.dram_tensor("h0_t", (128, KH, B), BF16, kind="ExternalInput")
    wpr_d = nc.dram_tensor("wpr8_t", (128, KH, VS), F8, kind="ExternalInput")
    if bx_nz:
        bx_d = nc.dram_tensor("bx_t", (128, GC), F32, kind="ExternalInput")
    if bhn_nz:
        bhn_d = nc.dram_tensor("bhn_t", (128, KH), F32, kind="ExternalInput")
    if bproj_nz:
        bpr_d = nc.dram_tensor("bproj_s", (1, VS), F32, kind="ExternalInput")
    out_d = nc.dram_tensor("out_lp", (NROW, VS), F32, kind="ExternalOutput")

    with tile.TileContext(nc) as tc:
        with tc.tile_pool(name="sb", bufs=1) as sb, \
             tc.tile_pool(name="ps", bufs=1, space="PSUM") as ps, \
             tc.tile_pool(name="dram", bufs=1, space="DRAM") as dp:

            # ---------- persistent loads / consts ---------------------------
            # recurrence-critical loads first; W_proj streams in later.
            whh_sb = sb.tile([128, KH, G], BF16)
            nc.sync.dma_start(whh_sb[:], whh_d[:])
            wih_sb = sb.tile([128, KE, G], F8)
            nc.sync.dma_start(wih_sb[:], wih_d[:])
            h0_sb = sb.tile([128, KH, B], BF16)
            nc.sync.dma_start(h0_sb[:], h0_d[:])
            if bx_nz:
                bx_sb = sb.tile([128, GC], F32)
                nc.sync.dma_start(bx_sb[:], bx_d[:])
            if bhn_nz:
                bhn_sb = sb.tile([128, KH], F32)
                nc.sync.dma_start(bhn_sb[:], bhn_d[:])

            ebias = sb.tile([128, 1], F32)
            nc.gpsimd.memset(ebias[:], EXP_BIAS)
            half = sb.tile([128, 1], F32)
            nc.gpsimd.memset(half[:], 0.5)
            S_all = sb.tile([128, NMT * NVU * 2], F32)   # exp partial sums
            nc.gpsimd.memset(S_all[:], 0.0)
            HT = sb.tile([128, S, KH, B], BF16)      # bf16 hidden states
            HT8 = sb.tile([128, S, KH, B], F8)       # fp8 mirror for phase 2

            # W_proj shard load: split so no single queue hogs the DMA ring.
            wpr_sb = sb.tile([128, KH, VS], F8)
            for kq in range(KH):
                nc.scalar.dma_start(wpr_sb[:, kq, :], wpr_d[:, kq, :])
            if bproj_nz:
                bpr_sb = sb.tile([128, VS], F32)
                nc.gpsimd.dma_start(bpr_sb[:],
                                    bpr_d[:1, :].to_broadcast([128, VS]))

            # ---------------- phase 0: chunk prep ---------------------------
            # rz PSUM bank layout per chunk: [128, 8, CH, B]  (r gc0-3, z gc0-3)
            # nx PSUM bank layout per chunk: [128, 4, CH, B]  (n gates x-side)
            rz_tiles = {}
            nx_tiles = {}
            embt_all = sb.tile([128, KE, NROW], BF16)
            emb8_all = sb.tile([128, KE, NROW], F8)
            gathered = set()

            def emit_gather(pair):
                # one 128-row gather + transpose + fp8 cast per TWO chunks
                lo = pair * 2 * CH * B
                nr = min(128, NROW - lo)
                idx_t = sb.tile([128, 1], I32, tag="idx", bufs=16,
                                name=f"idx{pair}")
                nc.sync.dma_start(idx_t[:nr], trg_d[lo:lo + nr, :])
                rows = sb.tile([128, E], BF16, tag="embr", bufs=16,
                               name=f"embr{pair}")
                nc.gpsimd.indirect_dma_start(
                    out=rows[:nr], out_offset=None, in_=tbl_d[:],
                    in_offset=bass.IndirectOffsetOnAxis(ap=idx_t[:nr, :1],
                                                        axis=0))
                for kb in range(KE):
                    nc.sync.dma_start_transpose(
                        embt_all[:, kb, lo:lo + nr],
                        rows[:nr, kb * 128:(kb + 1) * 128])
                nc.gpsimd.tensor_scalar(out=emb8_all[:, :, lo:lo + nr],
                                        in0=embt_all[:, :, lo:lo + nr],
                                        scalar1=EMB_S,
                                        scalar2=None, op0=OP.mult)

            def emit_prep(c):
                pair = c // 2
                if pair not in gathered:
                    gathered.add(pair)
                    emit_gather(pair)
                emb8 = emb8_all
                co = c * CH * B             # column offset in the flat array
                tlo = c * CH
                nst = min(CH, S - tlo)
                ncol = nst * B
                rz = ps.tile([128, 8, CH, B], F32, tag="rz", bufs=2,
                             name=f"rz{c}")
                nx = ps.tile([128, 4, CH, B], F32, tag="nx", bufs=1,
                             name=f"nx{c}")
                rz_tiles[c] = rz
                nx_tiles[c] = nx
                # fp8 DoubleRow x-gate matmuls straight into the psum banks.
                # rz group stays open; closed by the last W_hh matmul of the
                # chunk's last step (emit_step).
                for gc8 in range(8):
                    nc.tensor.matmul(
                        rz[:, gc8, :nst, :],
                        lhsT=wih_sb[:, :, gc8 * 128:(gc8 + 1) * 128],
                        rhs=emb8[:, :, co:co + ncol],
                        start=(gc8 == 0), stop=False, perf_mode=DR,
                        skip_group_check=True)
                for gc4 in range(4):
                    nc.tensor.matmul(
                        nx[:, gc4, :nst, :],
                        lhsT=wih_sb[:, :, (8 + gc4) * 128:(9 + gc4) * 128],
                        rhs=emb8[:, :, co:co + ncol],
                        start=(gc4 == 0), stop=(gc4 == 3), perf_mode=DR,
                        skip_group_check=True)
                if bx_nz:
                    nc.vector.tensor_tensor(
                        out=rz[:, :, :nst, :], in0=rz[:, :, :nst, :],
                        in1=bx_sb[:, 0:8, None, None].to_broadcast(
                            [128, 8, nst, B]), op=OP.add)
                    nc.vector.tensor_tensor(
                        out=nx[:, :, :nst, :], in0=nx[:, :, :nst, :],
                        in1=bx_sb[:, 8:12, None, None].to_broadcast(
                            [128, 4, nst, B]), op=OP.add)

            # ---------------- phase 1: one recurrence step -------------------
            def emit_step(t):
                c, tl = t // CH, t % CH
                last_in_chunk = (tl == CH - 1) or (t == S - 1)
                rz = rz_tiles[c]
                nx = nx_tiles[c]
                h_prev = h0_sb[:, :, :] if t == 0 else HT[:, t - 1, :, :]
                # r gates first (shortest path to the n-chain), then z, n.
                for gc in range(4):
                    for kt in range(KH):
                        nc.tensor.matmul(
                            rz[:, gc, tl, :],
                            lhsT=whh_sb[:, kt, gc * 128:(gc + 1) * 128],
                            rhs=h_prev[:, kt, :],
                            start=False, stop=False,
                            skip_group_check=True)
                for gc in range(4):
                    for kt in range(KH):
                        stop = last_in_chunk and gc == 3 and kt == KH - 1
                        nc.tensor.matmul(
                            rz[:, 4 + gc, tl, :],
                            lhsT=whh_sb[:, kt, (4 + gc) * 128:(5 + gc) * 128],
                            rhs=h_prev[:, kt, :],
                            start=False, stop=stop,
                            skip_group_check=True)
                psn = ps.tile([128, 4, B], F32, tag="psn", bufs=1,
                              name=f"psn{t}")
                for gc in range(4):
                    for kt in range(KH):
                        nc.tensor.matmul(
                            psn[:, gc, :],
                            lhsT=whh_sb[:, kt, (8 + gc) * 128:(9 + gc) * 128],
                            rhs=h_prev[:, kt, :],
                            start=(gc == 0 and kt == 0),
                            stop=(gc == 3 and kt == KH - 1))
                # gates: sigma(x) = 0.5*tanh(x/2) + 0.5 ; preacts are 64x.
                rt = sb.tile([128, 4, B], BF16, tag="rt", bufs=2, name=f"rt{t}")
                nc.scalar.activation(rt[:], rz[:, 0:4, tl, :], AF.Tanh,
                                     scale=0.5 / WS)
                zt = sb.tile([128, 4, B], BF16, tag="zt", bufs=2, name=f"zt{t}")
                nc.scalar.activation(zt[:], rz[:, 4:8, tl, :], AF.Tanh,
                                     scale=0.5 / WS)
                # q = 1-z on ACT (idle between zt and n tanh)
                q_s = sb.tile([128, 4, B], BF16, tag="q_s", bufs=2,
                              name=f"qs{t}")
                nc.scalar.activation(q_s[:], zt[:], AF.Identity,
                                     bias=half[:, :1], scale=-0.5)
                r_s = sb.tile([128, 4, B], BF16, tag="r_s", bufs=2,
                              name=f"rs{t}")
                nc.vector.tensor_scalar(out=r_s[:], in0=rt[:], scalar1=0.5,
                                        scalar2=0.5, op0=OP.mult, op1=OP.add)
                # n gate: narg = 64*(xn + r*hn)
                if bhn_nz:
                    nc.vector.tensor_tensor(
                        out=psn[:], in0=psn[:],
                        in1=bhn_sb[:, :, None].to_broadcast([128, 4, B]),
                        op=OP.add)
                tmpn = sb.tile([128, 4, B], BF16, tag="tmpn", bufs=2,
                               name=f"tn{t}")
                nc.vector.tensor_tensor(out=tmpn[:], in0=psn[:], in1=r_s[:],
                                        op=OP.mult)
                narg = sb.tile([128, 4, B], BF16, tag="narg", bufs=2,
                               name=f"na{t}")
                nc.vector.tensor_tensor(out=narg[:], in0=tmpn[:],
                                        in1=nx[:, :, tl, :], op=OP.add)
                # z path on DVE (fits between the n-chain ops)
                z_s = sb.tile([128, 4, B], BF16, tag="z_s", bufs=2,
                              name=f"zs{t}")
                nc.vector.tensor_scalar(out=z_s[:], in0=zt[:], scalar1=0.5,
                                        scalar2=0.5, op0=OP.mult, op1=OP.add)
                p_s = sb.tile([128, 4, B], BF16, tag="p_s", bufs=2,
                              name=f"ps{t}")
                nc.vector.tensor_tensor(out=p_s[:], in0=z_s[:], in1=h_prev,
                                        op=OP.mult)
                n_s = sb.tile([128, 4, B], BF16, tag="n_s", bufs=2,
                              name=f"ns{t}")
                nc.scalar.activation(n_s[:], narg[:], AF.Tanh, scale=1.0 / WS)
                # h' = n*(1-z) + z*h  (contiguous step-major writes)
                w_s = sb.tile([128, 4, B], BF16, tag="w_s", bufs=2,
                              name=f"ws{t}")
                nc.vector.tensor_tensor(out=w_s[:], in0=n_s[:], in1=q_s[:],
                                        op=OP.mult)
                nc.vector.tensor_tensor(out=HT[:, t, :, :], in0=w_s[:],
                                        in1=p_s[:], op=OP.add)
                nc.vector.tensor_copy(HT8[:, t, :, :], HT[:, t, :, :])

            # ---------------- phase 2 emission helpers ----------------------
            logit_tiles = {}
            lse_tiles = {}
            pl_tiles = {}

            def emit_munit_mm(m, u):
                mp = min(128, NROW - m * 128)
                t0 = (m * 128) // B
                nt = (mp + B - 1) // B
                if u == 0:
                    logit_tiles[m] = sb.tile([128, VS], F16, tag="logit",
                                             bufs=12, name=f"lg{m}")
                pl = ps.tile([128, 2, 512], F32, tag="pl", bufs=2,
                             name=f"pl{m}_{u}")
                pl_tiles[(m, u)] = pl
                lhsT = HT8[:, t0:t0 + nt, :, :].rearrange(
                    "p t k b -> p k t b")
                for hf in range(2):
                    v0 = u * VU + hf * 500
                    for kp in range(2):
                        nc.tensor.matmul(
                            pl[:mp, hf, :500],
                            lhsT=lhsT[:, 2 * kp:2 * kp + 2, :, :],
                            rhs=wpr_sb[:, 2 * kp:2 * kp + 2, v0:v0 + 500],
                            start=(kp == 0), stop=(kp == 1), perf_mode=DR)

            def emit_munit_post(m, u):
                mp = min(128, NROW - m * 128)
                pl = pl_tiles.pop((m, u))
                lg = logit_tiles[m]
                if bproj_nz:
                    for hf in range(2):
                        v0 = u * VU + hf * 500
                        nc.vector.tensor_tensor(
                            out=pl[:mp, hf, :500], in0=pl[:mp, hf, :500],
                            in1=bpr_sb[:mp, v0:v0 + 500], op=OP.add)
                esc = sb.tile([128, 2, 500], F16, tag="exps", bufs=2,
                              name=f"esc{m}_{u}")
                for hf in range(2):
                    k = (m * NVU + u) * 2 + hf
                    nc.scalar.activation(
                        esc[:mp, hf], pl[:mp, hf, :500], AF.Exp,
                        bias=ebias[:mp, :1], scale=1.0 / WS,
                        accum_out=S_all[:mp, k:k + 1])
                    nc.vector.tensor_scalar(
                        out=lg[:mp, u * VU + hf * 500:u * VU + hf * 500 + 500],
                        in0=pl[:mp, hf, :500], scalar1=1.0 / WS, scalar2=None,
                        op0=OP.mult)

            def emit_group_sums(gi):
                m0, nm = GROUPS[gi]
                sg = sb.tile([128, 16], F32, tag="sg", bufs=2, name=f"sg{gi}")
                for j in range(nm):
                    m = m0 + j
                    nc.vector.reduce_sum(
                        out=sg[:, j:j + 1],
                        in_=S_all[:, m * NVU * 2:(m + 1) * NVU * 2],
                        axis=mybir.AxisListType.X)
                cin = dp.tile([128, nm], F32, tag=f"cin{gi}", name=f"cin{gi}")
                nc.gpsimd.dma_start(cin[:], sg[:, :nm])
                return cin

            def emit_group_allreduce(gi, cin):
                nm = GROUPS[gi][1]
                cout = dp.tile([128, nm], F32, tag=f"cout{gi}",
                               addr_space="Shared", name=f"cout{gi}")
                nc.gpsimd.collective_compute(
                    "AllReduce", OP.add,
                    replica_groups=[list(range(NCORES))],
                    ins=[cin.opt()], outs=[cout.opt()])
                return cout

            def emit_group_lse(gi, cout):
                m0, nm = GROUPS[gi]
                st = sb.tile([128, 16], F32, tag="st", bufs=2, name=f"st{gi}")
                nc.gpsimd.dma_start(st[:, :nm], cout[:])
                # neg_lse = -(e - 127 + 4) * ln2 - ln(m),  St = m * 2^(e-127)
                iu = st[:, :nm].bitcast(U32)
                eu = sb.tile([128, 16], U32, tag="eu", bufs=2, name=f"eu{gi}")
                nc.vector.tensor_scalar(out=eu[:, :nm], in0=iu, scalar1=23,
                                        scalar2=None,
                                        op0=OP.logical_shift_right)
                ef = sb.tile([128, 16], F32, tag="ef", bufs=2, name=f"ef{gi}")
                nc.vector.tensor_copy(ef[:, :nm], eu[:, :nm])
                mu = sb.tile([128, 16], U32, tag="mu", bufs=2, name=f"mu{gi}")
                nc.vector.tensor_scalar(out=mu[:, :nm], in0=iu,
                                        scalar1=0x007FFFFF,
                                        scalar2=0x3F800000,
                                        op0=OP.bitwise_and, op1=OP.bitwise_or)
                mf = mu[:, :nm].bitcast(F32)
                acc = sb.tile([128, 16], F32, tag="acc", bufs=2,
                              name=f"acc{gi}")
                cfs = _NEGLN_COEF
                nc.vector.tensor_scalar(out=acc[:, :nm], in0=mf,
                                        scalar1=cfs[0], scalar2=cfs[1],
                                        op0=OP.mult, op1=OP.add)
                for k in range(2, 6):
                    nc.vector.tensor_tensor(out=acc[:, :nm], in0=acc[:, :nm],
                                            in1=mf, op=OP.mult)
                    nc.vector.tensor_scalar(out=acc[:, :nm], in0=acc[:, :nm],
                                            scalar1=cfs[k], scalar2=None,
                                            op0=OP.add)
                e2 = sb.tile([128, 16], F32, tag="e2", bufs=2, name=f"e2{gi}")
                nc.vector.tensor_scalar(out=e2[:, :nm], in0=ef[:, :nm],
                                        scalar1=-LN2,
                                        scalar2=(127.0 - 4.0) * LN2,
                                        op0=OP.mult, op1=OP.add)
                nlse = sb.tile([128, 16], F32, tag="nlse", bufs=2,
                               name=f"nlse{gi}")
                nc.vector.tensor_tensor(out=nlse[:, :nm], in0=acc[:, :nm],
                                        in1=e2[:, :nm], op=OP.add)
                lse_tiles[gi] = nlse

            out_tiles = {}

            def emit_out_piece(m, piece, eng=0):
                # piece = 500 cols; DMA fires per 2000 cols on the scalar
                # queue (hwdge)
                gi = next(i for i, (m0, nm) in enumerate(GROUPS)
                          if m0 <= m < m0 + nm)
                j = m - GROUPS[gi][0]
                mp = min(128, NROW - m * 128)
                nlse = lse_tiles[gi]
                lg = logit_tiles[m]
                hf = piece // 4
                if piece % 4 == 0:
                    out_tiles[m] = sb.tile([128, 2000], F32, tag="ot",
                                           bufs=3, name=f"ot{m}_{hf}")
                ot = out_tiles[m]
                c0 = (piece % 4) * 500
                if eng == 2:
                    nc.scalar.activation(
                        ot[:mp, c0:c0 + 500],
                        lg[:mp, hf * 2000 + c0:hf * 2000 + c0 + 500],
                        AF.Identity, bias=nlse[:mp, j:j + 1], scale=1.0)
                else:
                    e = nc.vector if eng == 1 else nc.gpsimd
                    e.tensor_tensor(
                        out=ot[:mp, c0:c0 + 500],
                        in0=lg[:mp, hf * 2000 + c0:hf * 2000 + c0 + 500],
                        in1=nlse[:mp, j:j + 1].to_broadcast([mp, 500]),
                        op=OP.add)
                if piece % 4 == 3:
                    nc.scalar.dma_start(
                        out_d[m * 128:m * 128 + mp,
                              hf * 2000:(hf + 1) * 2000], ot[:mp])
                    if piece == 7:
                        logit_tiles.pop(m)

            # ---------------- main emission loop ----------------------------
            from collections import deque
            work_q = deque()
            deferred = {}
            cur_step = [0]

            def defer(steps, fn):
                tgt = cur_step[0] + steps
                if tgt >= S:
                    work_q.append(fn)   # lands in the final drain
                else:
                    deferred.setdefault(tgt, []).append(fn)

            def enqueue_mtile(m):
                for u in range(NVU):
                    work_q.append(lambda m=m, u=u: emit_munit_mm(m, u))
                    work_q.append(lambda m=m, u=u: emit_munit_post(m, u))
                for gi, (m0, nm) in enumerate(GROUPS):
                    if m == m0 + nm - 1:
                        def sums(gi=gi):
                            cin = emit_group_sums(gi)

                            def issue(gi=gi, cin=cin):
                                cout = emit_group_allreduce(gi, cin)
                                delay = GROUP_DELAY.get(
                                    gi, GROUP_DELAY_DEFAULT)

                                def fin(gi=gi, cout=cout):
                                    emit_group_lse(gi, cout)
                                    m0, nm = GROUPS[gi]
                                    for mm in range(m0, m0 + nm):
                                        for p in range(8):
                                            work_q.append(
                                                lambda mm=mm, p=p, eng=(0, 1, 2, 0)[p % 4]:
                                                emit_out_piece(mm, p, eng))
                                defer(delay, fin)
                            work_q.append(issue)
                        work_q.append(sums)

            # ALL chunk preps are emitted before any collective: hwdge
            # DMAs (idx loads, transposes) issued after a collective_compute
            # are held until the collective completes, so none may follow
            # one in program order.  The scheduler orders execution by
            # readiness, so this costs nothing.
            for c in range(NCH):
                emit_prep(c)

            for t in range(S):
                cur_step[0] = t
                for fn in deferred.pop(t, []):
                    fn()
                with tc.high_priority(offset=10 ** 6):
                    emit_step(t)
                if t >= 3 and (t - 3) % 4 == 0:
                    enqueue_mtile((t - 3) // 4)
                ndrain = 6 if t < S - 1 else len(work_q)
                for _ in range(min(ndrain, len(work_q))):
                    work_q.popleft()()
            cur_step[0] = S
            for t in sorted(deferred):
                for fn in deferred[t]:
                    fn()
            for m in range(((S - 1 - 3) // 4) + 1, NMT):
                enqueue_mtile(m)
            while work_q:
                work_q.popleft()()

    nc.finalize()
    _BUILD_CACHE[key] = nc
    return nc


def _pack_T(w, ktiles, scale=1.0, dtype=ml_dtypes.bfloat16):
    """[out, in] f32 -> [128, ktiles, out] (w.T, k-major slabs)."""
    wT = np.ascontiguousarray(w.T * scale).astype(dtype)
    return np.ascontiguousarray(
        wT.reshape(ktiles, 128, w.shape[0]).transpose(1, 0, 2))


LAST_PROFILE = None


def kernel(trg, h0, embed_table, W_ih, W_hh, b_ih, b_hh, W_proj, b_proj):
    global LAST_PROFILE
    trg = np.asarray(trg)
    h0 = np.asarray(h0, dtype=np.float32)
    embed_table = np.asarray(embed_table, dtype=np.float32)
    W_ih = np.asarray(W_ih, dtype=np.float32)
    W_hh = np.asarray(W_hh, dtype=np.float32)
    b_ih = np.asarray(b_ih, dtype=np.float32)
    b_hh = np.asarray(b_hh, dtype=np.float32)
    W_proj = np.asarray(W_proj, dtype=np.float32)
    b_proj = np.asarray(b_proj, dtype=np.float32)

    bx = b_ih.copy()
    bx[:2 * H] += b_hh[:2 * H]
    bhn = b_hh[2 * H:]
    bhn_nz = bool(np.any(bhn))
    bx_nz = bool(np.any(bx))
    bproj_nz = bool(np.any(b_proj))
    nc = _build(bhn_nz, bx_nz, bproj_nz)

    # host-side layout prep (sharding/packing only)
    trg_flat = np.ascontiguousarray(
        trg[:, :S].T.reshape(NROW, 1)).astype(np.int32)
    tbl_bf = embed_table.astype(ml_dtypes.bfloat16)
    f8 = ml_dtypes.float8_e4m3
    h0T = np.ascontiguousarray(
        h0[0].T.reshape(KH, 128, B).transpose(1, 0, 2))

    base = {
        "trg_flat": trg_flat,
        "emb_tbl": tbl_bf,
        "wih8_t": _pack_T(W_ih, KE, scale=W_IH_S, dtype=f8),
        "whh_t": _pack_T(W_hh, KH, scale=WS),
        "h0_t": h0T.astype(ml_dtypes.bfloat16),
    }
    if bx_nz:
        base["bx_t"] = np.ascontiguousarray(
            (bx * WS).reshape(GC, 128).T).astype(np.float32)
    if bhn_nz:
        base["bhn_t"] = np.ascontiguousarray(
            (bhn * WS).reshape(KH, 128).T).astype(np.float32)

    in_maps = []
    for c in range(NCORES):
        m = dict(base)
        m["wpr8_t"] = _pack_T(W_proj[c * VS:(c + 1) * VS], KH, scale=WS,
                              dtype=f8)
        if bproj_nz:
            m["bproj_s"] = np.ascontiguousarray(
                b_proj[c * VS:(c + 1) * VS].reshape(1, VS) * WS)
        in_maps.append(m)

    trace = bool(int(os.environ.get("KERNEL_TRACE", "0")))
    res = run_bass_kernel_spmd(nc, in_maps, core_ids=list(range(NCORES)),
                               trace=trace)
    LAST_PROFILE = res

    out = np.zeros((B, T, V), dtype=np.float32)
    big = np.stack([res.results[c]["out_lp"].reshape(S, B, VS)
                    for c in range(NCORES)], axis=0)   # [c, t, b, vs]
    out[:, 1:, :] = big.transpose(2, 1, 0, 3).reshape(B, S, V)
    return out

